# revision 7
# baseline (speedup 1.0000x reference)
"""Trainium2 Bass kernel for nn_BagleyDiT (DiT block: AdaLN + self-attn with
RoPE + cross-attn + top-2 MoE FFN), SPMD across 8 NeuronCores.

Sharding:
  - tokens: core c owns positions [128c, 128c+128) of BOTH batches (256 rows)
  - attention: head-parallel (2 of 16 heads per core); activations exchanged
    via AllGather in feature-major bf16
  - AdaLN modulation: contraction-sharded matmul + AllReduce
  - MoE: expert-parallel second launch (core c = expert c), host routes.

Activation layout conventions:
  token-major  [tokens(part), feat(free)] for LN / residual / pointwise
  feature-major[feat(part), tokens(free)] for matmul operands; global token
  column order is (chunk c', batch b, pos j) -> col 256c' + 128b + j.

SBUF/PSUM pools are statically allocated per tag, so tags are deliberately
aliased across phases with disjoint lifetimes.
"""

import os
import sys
import threading
from contextlib import ExitStack

import numpy as np

sys.path.insert(0, "/opt/trn_rl_repo")

import concourse.bass as bass  # noqa: E402
import concourse.mybir as mybir  # noqa: E402
import concourse.tile as tile  # noqa: E402
from concourse import bacc  # noqa: E402
from concourse.bass import ds  # noqa: E402
from concourse.bass_utils import run_bass_kernel_spmd  # noqa: E402
from concourse.masks import make_identity  # noqa: E402

B, S, H, NH, HD = 2, 1024, 1024, 16, 64
L, C, E, TOPK, D = 256, 2048, 8, 2, 2048
EPS = 1e-5
NC = 8
T = B * S
CAP = 768       # launch-2 tokens per expert per batch

F32 = mybir.dt.float32
BF16 = mybir.dt.bfloat16
AF = mybir.ActivationFunctionType
ALU = mybir.AluOpType

RG = [list(range(NC))]

_cache = {}
_lock = threading.Lock()


# ---------------------------------------------------------------- helpers
def _ln_tok(nc, pool, x_t, eps_t):
    """LayerNorm over free dim of token-major x_t [128, 1024] f32."""
    st = pool.tile([128, 2, 6], F32, tag="ln_st")
    mv = pool.tile([128, 2], F32, tag="ln_mv")
    nc.vector.bn_stats(out=st[:, 0, :], in_=x_t[:, 0:512])
    nc.vector.bn_stats(out=st[:, 1, :], in_=x_t[:, 512:1024])
    nc.vector.bn_aggr(out=mv[:], in_=st[:])
    rstd = pool.tile([128, 1], F32, tag="ln_rstd")
    nc.scalar.activation(out=rstd[:], in_=mv[:, 1:2], func=AF.Sqrt, bias=eps_t[:])
    nc.vector.reciprocal(out=rstd[:], in_=rstd[:])
    ln = pool.tile([128, 1024], F32, tag="ln_out")
    nc.vector.tensor_scalar(
        out=ln[:], in0=x_t[:], scalar1=mv[:, 0:1], scalar2=rstd[:],
        op0=ALU.subtract, op1=ALU.mult)
    return ln


def _transpose_to(nc, pspool, ident, src_t, dst_t, st):
    """PE-transpose token-major src_t [128, 1024] into feature-major dst_t
    [128, 8, 256] at [:, o, 128*st : +128] (casts to dst dtype on evict)."""
    for o in range(8):
        pt = pspool.tile([128, 128], F32, tag="psmisc", name=f"tr_ps{o}")
        nc.tensor.transpose(pt[:], src_t[:, o * 128:(o + 1) * 128], ident[:])
        nc.scalar.activation(
            out=dst_t[:, o, st * 128:(st + 1) * 128], in_=pt[:], func=AF.Copy)


def _bcast_from_mod(nc, pool, mod_dram, b, seg, add_one, tag):
    """DMA-broadcast mod[b, 1024*seg : +1024] to all partitions [128,1024]."""
    t = pool.tile([128, 1024], F32, tag=tag, name=f"bc_{tag}_{b}_{seg}")
    src = mod_dram[:]
    ap = bass.AP(
        tensor=src.tensor,
        offset=src.offset + b * 6 * H + seg * H,
        ap=[[0, 128], [1, 1024]],
    )
    nc.sync.dma_start(out=t[:], in_=ap)
    if add_one:
        nc.vector.tensor_scalar_add(t[:], t[:], 1.0)
    return t


def _attention(nc, work, psum, qT, kT, v_sb, outT, n_tkblocks, tkb_of,
               k_is_flat=False):
    """Head-sharded attention for 2 local heads x 2 batches.

    qT [128, 2048] bf16, cols (c' 8, b 2, j 128). kT: same layout (self) or
    [128, 512] cols (b*256 + l) (cross). v_sb [128, ntb, 2, 65] token-major
    with a ones column at 64 (gives softmax row-sums as av row 64).
    outT [128, 2048] bf16 feature-major, same col order as qT.
    Softmax = plain exp (scores bounded by construction; no max-sub)."""
    q4 = qT.rearrange("p (c s j) -> p c s j", c=8, s=2)
    o4 = outT.rearrange("p (c s j) -> p c s j", c=8, s=2)
    for b in range(2):
        for hh in range(2):
            hsl = slice(hh * 64, hh * 64 + 64)
            av = [psum.tile([128, 512], F32, tag=f"av{nt}", bufs=1,
                            name=f"av{nt}_{b}_{hh}")
                  for nt in range(2)]
            for mt in range(n_tkblocks):
                if k_is_flat:
                    lhsK = kT[hsl, b * 256 + mt * 128: b * 256 + (mt + 1) * 128]
                else:
                    lhsK = kT.rearrange(
                        "p (c s j) -> p c s j", c=8, s=2)[hsl, mt, b, :]
                for nt in range(2):
                    sp = psum.tile([128, 512], F32, tag="ps512",
                                   name=f"s_ps{nt}")
                    nc.tensor.matmul(
                        sp[:], lhsK, q4[hsl, 4 * nt:4 * nt + 4, b, :],
                        start=True, stop=True)
                    p_sb = work.tile([128, 512], BF16, tag="p_sb", bufs=3)
                    nc.scalar.activation(out=p_sb[:], in_=sp[:], func=AF.Exp)
                    nc.tensor.matmul(
                        av[nt][0:65, :], v_sb[:, tkb_of(b, mt), hh, :],
                        p_sb[:], start=(mt == 0), stop=(mt == n_tkblocks - 1))
            rec = work.tile([1, 1024], F32, tag="rec", bufs=1)
            for nt in range(2):
                nc.vector.reciprocal(
                    out=rec[:, nt * 512:(nt + 1) * 512], in_=av[nt][64:65, :])
            rb = work.tile([64, 1024], F32, tag="rb", bufs=1)
            nc.gpsimd.partition_broadcast(rb[:], rec[:])
            for nt in range(2):
                nc.vector.tensor_mul(
                    o4[hsl, 4 * nt:4 * nt + 4, b, :],
                    av[nt][0:64, :].rearrange("p (c j) -> p c j", c=4),
                    rb[:, nt * 512:(nt + 1) * 512].rearrange(
                        "p (c j) -> p c j", c=4))


def _proj_residual(nc, work, psum, big, bcpool, pid, agT, w_in, b_bc, xin_t,
                   gate, tag, out_tag):
    """out = xin + [gate_b *] (own_chunk(agT) @ W + bias), token-major.
    agT: DRAM [1024, 2048] bf16 (rows=feat, cols=(c',b,j)); own chunk cols =
    [256*pid : +256]. Returns 2 tiles [128, 1024] f32 (st = batch)."""
    w_sb = big.tile([128, 8, 1024], BF16, tag="wbig", bufs=2,
                    name=f"w_{tag}")
    nc.sync.dma_start(w_sb[:], w_in.ap().rearrange("(o p) m -> p o m", p=128))
    bias_sb = work.tile([128, 1024], F32, tag="bias_pr", bufs=2,
                        name=f"bias_{tag}")
    nc.sync.dma_start(bias_sb[:], b_bc.ap())
    aT_own = big.tile([128, 8, 256], BF16, tag="aown", bufs=2,
                      name=f"aown_{tag}")
    nc.sync.dma_start(
        aT_own[:],
        agT[:].rearrange("(k p) t -> p k t", p=128)[:, :, ds(pid * 256, 256)])
    out_t = [None, None]
    for st in range(2):
        out_t[st] = big.tile([128, 1024], F32, tag=f"{out_tag}_{st}", bufs=1,
                             name=f"out_{tag}_{st}")
        g_bc = None
        if gate is not None:
            g_bc = _bcast_from_mod(nc, bcpool, gate[0], st, gate[1], False,
                                   "bcG")
        for nt in range(2):
            pp = psum.tile([128, 512], F32, tag="ps512", name=f"proj_ps{nt}")
            for k in range(8):
                nc.tensor.matmul(
                    pp[:], aT_own[:, k, st * 128:(st + 1) * 128],
                    w_sb[:, k, nt * 512:(nt + 1) * 512],
                    start=(k == 0), stop=(k == 7))
            sl = slice(nt * 512, (nt + 1) * 512)
            tmp = work.tile([128, 512], F32, tag="prj_tmp")
            nc.vector.tensor_add(tmp[:], pp[:], bias_sb[:, sl])
            if g_bc is not None:
                nc.vector.tensor_mul(tmp[:], tmp[:], g_bc[:, sl])
            nc.vector.tensor_add(out_t[st][:, sl], tmp[:], xin_t[st][:, sl])
    return out_t


# ---------------------------------------------------------------- launch 1
def build_launch1():
    nc = bacc.Bacc("TRN2", target_bir_lowering=False, debug=False,
                   num_devices=NC)

    def din(name, shape, dt=F32):
        return nc.dram_tensor(name, shape, dt, kind="ExternalInput")

    xch = din("xch", [256, 1024])
    condTs = din("condTs", [128, 2])
    adaln_ws = din("adaln_ws", [128, 6144], BF16)
    adaln_b8 = din("adaln_b8", [2, 6144])
    qwh = din("qwh", [1024, 128], BF16)
    kwh = din("kwh", [1024, 128], BF16)
    vwh = din("vwh", [1024, 128], BF16)
    qbh = din("qbh", [128, 1])
    kbh = din("kbh", [128, 1])
    vbh_bc = din("vbh_bc", [128, 128])
    cosQ = din("cosQ", [128, 2048], BF16)
    sinQ = din("sinQ", [128, 2048], BF16)
    perm128 = din("perm128", [128, 128], BF16)
    ow_in = din("ow_in", [1024, 1024], BF16)
    ob_bc = din("ob_bc", [128, 1024])
    encT = din("encT", [2048, 512], BF16)
    cqwh = din("cqwh", [1024, 128], BF16)
    cqbh = din("cqbh", [128, 1])
    ckwh = din("ckwh", [2048, 128], BF16)
    ckbh = din("ckbh", [128, 1])
    cvwh = din("cvwh", [2048, 128], BF16)
    cvbh_bc = din("cvbh_bc", [128, 128])
    cow_in = din("cow_in", [1024, 1024], BF16)
    cob_bc = din("cob_bc", [128, 1024])
    cng_bc = din("cng_bc", [128, 1024])
    cnb_bc = din("cnb_bc", [128, 1024])
    gate_w_in = din("gate_w_in", [1024, 8])

    x2_out = nc.dram_tensor("x2_out", [256, 1024], F32, kind="ExternalOutput")
    h2_out = nc.dram_tensor("h2_out", [256, 1024], F32, kind="ExternalOutput")
    logitsT_out = nc.dram_tensor("logitsT_out", [8, 256], F32,
                                 kind="ExternalOutput")
    mod_out = nc.dram_tensor("mod_out", [2, 6144], F32, kind="ExternalOutput")

    with tile.TileContext(nc) as tc, ExitStack() as ctx:
        const = ctx.enter_context(tc.tile_pool(name="const", bufs=1))
        big = ctx.enter_context(tc.tile_pool(name="big", bufs=1))
        work = ctx.enter_context(tc.tile_pool(name="work", bufs=2))
        bcpool = ctx.enter_context(tc.tile_pool(name="bcast", bufs=1))
        psum = ctx.enter_context(tc.tile_pool(name="psum", bufs=2, space="PSUM"))
        dram = ctx.enter_context(tc.tile_pool(name="dram", bufs=1, space="DRAM"))

        ident = const.tile([128, 128], F32)
        make_identity(nc, ident)
        eps_t = const.tile([128, 1], F32)
        nc.vector.memset(eps_t, EPS)

        pid = nc.sync.partition_id()

        # -------- Phase 0: AdaLN modulation (K-sharded matmul + AllReduce)
        cond_sb = work.tile([128, 2], F32, tag="cond")
        nc.sync.dma_start(cond_sb[:], condTs.ap())
        silu_bf = work.tile([128, 2], BF16, tag="silu")
        nc.scalar.activation(out=silu_bf[:], in_=cond_sb[:], func=AF.Silu)
        modAR_in = dram.tile([2, 6144], F32)
        modAR = dram.tile([2, 6144], F32, addr_space="Shared")
        for j in range(12):
            jsl = slice(j * 512, (j + 1) * 512)
            aw_j = work.tile([128, 512], BF16, tag="aw_j", bufs=3,
                             name=f"aw_{j}")
            nc.sync.dma_start(aw_j[:], adaln_ws.ap()[:, jsl])
            bp = work.tile([2, 512], F32, tag="biasp", name=f"bp_{j}")
            nc.sync.dma_start(bp[:], adaln_b8.ap()[:, jsl])
            mp = psum.tile([2, 512], F32, tag="ps512", name=f"mod_ps{j}")
            nc.tensor.matmul(mp[:], silu_bf[:], aw_j[:],
                             start=True, stop=True)
            mo = work.tile([2, 512], F32, tag="modp", name=f"mo_{j}")
            nc.vector.tensor_add(out=mo[:], in0=mp[:], in1=bp[:])
            nc.sync.dma_start(modAR_in[:, jsl], mo[:])
        nc.gpsimd.collective_compute(
            "AllReduce", ALU.add, replica_groups=RG,
            ins=[modAR_in.opt()], outs=[modAR.opt()])
        nc.sync.dma_start(mod_out.ap(), modAR[:])

        # -------- Phase 1: h1 = ln(x)*(1+sc1)+sh1 -> transpose -> AllGather
        x_t = [None, None]
        for st in range(2):
            x_t[st] = big.tile([128, 1024], F32, tag=f"x_{st}", bufs=1,
                               name=f"x_{st}")
            nc.sync.dma_start(x_t[st][:], xch.ap()[st * 128:(st + 1) * 128, :])

        hT = work.tile([128, 8, 256], BF16, tag="hT", bufs=1)
        for st in range(2):
            ln = _ln_tok(nc, work, x_t[st], eps_t)
            A1 = _bcast_from_mod(nc, bcpool, modAR, st, 1, True, "bcA")
            B1 = _bcast_from_mod(nc, bcpool, modAR, st, 0, False, "bcB")
            h1 = work.tile([128, 1024], F32, tag="h1")
            nc.vector.tensor_mul(h1[:], ln[:], A1[:])
            nc.vector.tensor_add(h1[:], h1[:], B1[:])
            _transpose_to(nc, psum, ident, h1, hT, st)
        hAG_in = dram.tile([1024, 256], BF16)
        hAG = dram.tile([8192, 256], BF16, addr_space="Shared")
        nc.sync.dma_start(hAG_in[:].rearrange("(o p) t -> p o t", p=128), hT[:])
        nc.gpsimd.collective_compute(
            "AllGather", ALU.bypass, replica_groups=RG,
            ins=[hAG_in.opt()], outs=[hAG.opt()])

        # -------- Phase 2: QKV projections (+RoPE on q, k)
        qw_sb = const.tile([128, 8, 128], BF16, tag="qw")
        kw_sb = const.tile([128, 8, 128], BF16, tag="kw")
        vw_sb = const.tile([128, 8, 128], BF16, tag="vw")
        nc.sync.dma_start(qw_sb[:], qwh.ap().rearrange("(o p) m -> p o m", p=128))
        nc.sync.dma_start(kw_sb[:], kwh.ap().rearrange("(o p) m -> p o m", p=128))
        nc.sync.dma_start(vw_sb[:], vwh.ap().rearrange("(o p) m -> p o m", p=128))
        qb_sb = const.tile([128, 1], F32, tag="qb")
        kb_sb = const.tile([128, 1], F32, tag="kb")
        vb_sb = const.tile([128, 128], F32, tag="vb")
        nc.sync.dma_start(qb_sb[:], qbh.ap())
        nc.sync.dma_start(kb_sb[:], kbh.ap())
        nc.sync.dma_start(vb_sb[:], vbh_bc.ap())

        q_raw = big.tile([128, 2048], BF16, tag="t2048a", bufs=1, name="q_raw")
        k_raw = big.tile([128, 2048], BF16, tag="t2048b", bufs=1, name="k_raw")
        v_sb = big.tile([128, 16, 2, 65], BF16, tag="v_sb", bufs=1)
        nc.vector.memset(v_sb[:, :, :, 64:65], 1.0)

        hAG_r = hAG[:].rearrange("(c k p) t -> p k c t", p=128, k=8)

        def qkv_pass(wq, bq, wk, wv, src_r, dst_q, dst_k, make_v):
            for half in range(2):
                csl = slice(4 * half, 4 * half + 4)
                hk = []
                for k in range(8):
                    t = big.tile([128, 4, 256], BF16, tag=f"hT_k{k}", bufs=1,
                                 name=f"hT_k{k}_{half}")
                    nc.sync.dma_start(t[:], src_r[:, k, csl, :])
                    hk.append(t)
                for tg2 in range(2):
                    tg = 2 * half + tg2
                    qp = psum.tile([128, 512], F32, tag="ps512",
                                   name=f"q_ps{tg}")
                    kp = None
                    if wk is not None:
                        kp = psum.tile([128, 512], F32, tag="ps512",
                                       name=f"k_ps{tg}")
                    for k in range(8):
                        rhs = hk[k][:, 2 * tg2:2 * tg2 + 2, :]
                        nc.tensor.matmul(qp[:], wq[:, k, :], rhs,
                                         start=(k == 0), stop=(k == 7))
                        if kp is not None:
                            nc.tensor.matmul(kp[:], wk[:, k, :], rhs,
                                             start=(k == 0), stop=(k == 7))
                    nc.scalar.activation(
                        out=dst_q[:, tg * 512:(tg + 1) * 512], in_=qp[:],
                        func=AF.Identity, bias=bq[:])
                    if kp is not None:
                        nc.scalar.activation(
                            out=dst_k[:, tg * 512:(tg + 1) * 512], in_=kp[:],
                            func=AF.Identity, bias=kb_sb[:])
                if make_v:
                    for tb2 in range(8):
                        tb = 8 * half + tb2
                        c2, st_ = tb2 // 2, tb2 % 2
                        vp = psum.tile([128, 128], F32, tag="psmisc",
                                       name=f"v_ps{tb}")
                        for k in range(8):
                            nc.tensor.matmul(
                                vp[:], hk[k][:, c2, st_ * 128:(st_ + 1) * 128],
                                wv[:, k, :], start=(k == 0), stop=(k == 7))
                        for hh in range(2):
                            nc.vector.tensor_add(
                                out=v_sb[:, tb, hh, 0:64],
                                in0=vp[:, hh * 64:hh * 64 + 64],
                                in1=vb_sb[:, hh * 64:hh * 64 + 64])

        qkv_pass(qw_sb, qb_sb, kw_sb, vw_sb, hAG_r, q_raw, k_raw, True)

        perm_sb = const.tile([128, 128], BF16, tag="perm")
        nc.sync.dma_start(perm_sb[:], perm128.ap())
        cos_sb = big.tile([128, 2048], BF16, tag="cos", bufs=1)
        sin_sb = big.tile([128, 2048], BF16, tag="sin", bufs=1)
        nc.sync.dma_start(cos_sb[:], cosQ.ap())
        nc.sync.dma_start(sin_sb[:], sinQ.ap())
        q_r = big.tile([128, 2048], BF16, tag="t2048c", bufs=1, name="q_r")
        k_r = big.tile([128, 2048], BF16, tag="t2048d", bufs=1, name="k_r")
        for (src, dst) in ((q_raw, q_r), (k_raw, k_r)):
            for tg in range(4):
                sl = slice(tg * 512, (tg + 1) * 512)
                rp = psum.tile([128, 512], F32, tag="psmisc",
                               name=f"rot_ps{tg}")
                nc.tensor.matmul(rp[:], perm_sb[:], src[:, sl],
                                 start=True, stop=True)
                tmp = work.tile([128, 512], F32, tag="rope_tmp")
                nc.vector.tensor_mul(tmp[:], rp[:], sin_sb[:, sl])
                tmp2 = work.tile([128, 512], F32, tag="rope_tmp2")
                nc.vector.tensor_mul(tmp2[:], src[:, sl], cos_sb[:, sl])
                nc.vector.tensor_add(dst[:, sl], tmp[:], tmp2[:])

        # -------- Phase 3: self-attention + AllGather
        aT = big.tile([128, 2048], BF16, tag="t2048a", bufs=1, name="aT")
        _attention(nc, work, psum, q_r[:], k_r[:], v_sb, aT[:],
                   n_tkblocks=8, tkb_of=lambda b, mt: 2 * mt + b)
        aAG_in = dram.tile([128, 2048], BF16)
        aAG = dram.tile([1024, 2048], BF16, addr_space="Shared")
        nc.sync.dma_start(aAG_in[:], aT[:])
        nc.gpsimd.collective_compute(
            "AllGather", ALU.bypass, replica_groups=RG,
            ins=[aAG_in.opt()], outs=[aAG.opt()])

        # -------- Phase 4: out-proj + gated residual -> x1
        x1_t = _proj_residual(nc, work, psum, big, bcpool, pid, aAG, ow_in,
                              ob_bc, x_t, (modAR, 2), "x1", "x1")

        # -------- Phase 5: cross-attention
        hcT = work.tile([128, 8, 256], BF16, tag="hT", bufs=1, name="hcT")
        cng_sb = bcpool.tile([128, 1024], F32, tag="bcA", name="cng")
        nc.sync.dma_start(cng_sb[:], cng_bc.ap())
        cnb_sb = bcpool.tile([128, 1024], F32, tag="bcB", name="cnb")
        nc.sync.dma_start(cnb_sb[:], cnb_bc.ap())
        for st in range(2):
            ln = _ln_tok(nc, work, x1_t[st], eps_t)
            hc = work.tile([128, 1024], F32, tag="h1")
            nc.vector.tensor_mul(hc[:], ln[:], cng_sb[:])
            nc.vector.tensor_add(hc[:], hc[:], cnb_sb[:])
            _transpose_to(nc, psum, ident, hc, hcT, st)
        hcAG_in = dram.tile([1024, 256], BF16)
        hcAG = dram.tile([8192, 256], BF16, addr_space="Shared")
        nc.sync.dma_start(hcAG_in[:].rearrange("(o p) t -> p o t", p=128),
                          hcT[:])
        nc.gpsimd.collective_compute(
            "AllGather", ALU.bypass, replica_groups=RG,
            ins=[hcAG_in.opt()], outs=[hcAG.opt()])

        cqw_sb = const.tile([128, 8, 128], BF16, tag="cqw")
        nc.sync.dma_start(cqw_sb[:],
                          cqwh.ap().rearrange("(o p) m -> p o m", p=128))
        cqb_sb = const.tile([128, 1], F32, tag="cqb")
        nc.sync.dma_start(cqb_sb[:], cqbh.ap())
        lq = big.tile([128, 2048], BF16, tag="t2048c", bufs=1, name="lq")
        hcAG_r = hcAG[:].rearrange("(c k p) t -> p k c t", p=128, k=8)
        qkv_pass(cqw_sb, cqb_sb, None, None, hcAG_r, lq, None, False)

        ckw_sb = const.tile([128, 16, 128], BF16, tag="ckw")
        cvw_sb = const.tile([128, 16, 128], BF16, tag="cvw")
        nc.sync.dma_start(ckw_sb[:],
                          ckwh.ap().rearrange("(o p) m -> p o m", p=128))
        nc.sync.dma_start(cvw_sb[:],
                          cvwh.ap().rearrange("(o p) m -> p o m", p=128))
        ckb_sb = const.tile([128, 1], F32, tag="ckb")
        nc.sync.dma_start(ckb_sb[:], ckbh.ap())
        cvb_sb = const.tile([128, 128], F32, tag="cvb")
        nc.sync.dma_start(cvb_sb[:], cvbh_bc.ap())
        lk = work.tile([128, 512], BF16, tag="lk", bufs=1)
        lv = big.tile([128, 4, 2, 65], BF16, tag="lv", bufs=1)
        nc.vector.memset(lv[:, :, :, 64:65], 1.0)
        lkp = psum.tile([128, 512], F32, tag="av0", bufs=1, name="lk_ps")
        lv_tags = [("av1", 1), ("psmisc", 2), ("psmisc", 2), ("ps512", 2)]
        lvp = [psum.tile([128, 128], F32, tag=lv_tags[tb][0],
                         bufs=lv_tags[tb][1], name=f"lv_ps{tb}")
               for tb in range(4)]
        encT_r = encT.ap().rearrange("(o p) n -> p o n", p=128)
        for k in range(16):
            et = work.tile([128, 512], BF16, tag="encT_k", bufs=3,
                           name=f"encT_k{k}")
            nc.sync.dma_start(et[:], encT_r[:, k, :])
            nc.tensor.matmul(lkp[:], ckw_sb[:, k, :], et[:],
                             start=(k == 0), stop=(k == 15))
            for tb in range(4):
                nc.tensor.matmul(
                    lvp[tb][:], et[:, tb * 128:(tb + 1) * 128],
                    cvw_sb[:, k, :], start=(k == 0), stop=(k == 15))
        nc.scalar.activation(out=lk[:], in_=lkp[:], func=AF.Identity,
                             bias=ckb_sb[:])
        for tb in range(4):
            for hh in range(2):
                nc.vector.tensor_add(
                    out=lv[:, tb, hh, 0:64],
                    in0=lvp[tb][:, hh * 64:hh * 64 + 64],
                    in1=cvb_sb[:, hh * 64:hh * 64 + 64])

        caT = big.tile([128, 2048], BF16, tag="t2048b", bufs=1, name="caT")
        _attention(nc, work, psum, lq[:], lk[:], lv, caT[:],
                   n_tkblocks=2, tkb_of=lambda b, mt: 2 * b + mt,
                   k_is_flat=True)
        caAG_in = dram.tile([128, 2048], BF16)
        caAG = dram.tile([1024, 2048], BF16, addr_space="Shared")
        nc.sync.dma_start(caAG_in[:], caT[:])
        nc.gpsimd.collective_compute(
            "AllGather", ALU.bypass, replica_groups=RG,
            ins=[caAG_in.opt()], outs=[caAG.opt()])

        # -------- Phase 6: cow proj + residual -> x2
        x2_t = _proj_residual(nc, work, psum, big, bcpool, pid, caAG, cow_in,
                              cob_bc, x1_t, None, "x2", "x")
        for st in range(2):
            nc.sync.dma_start(x2_out.ap()[st * 128:(st + 1) * 128, :],
                              x2_t[st][:])

        # -------- Phase 7: h2 + logits
        h2T = work.tile([128, 8, 256], F32, tag="h2T", bufs=1)
        for st in range(2):
            ln = _ln_tok(nc, work, x2_t[st], eps_t)
            A2 = _bcast_from_mod(nc, bcpool, modAR, st, 4, True, "bcA")
            B2 = _bcast_from_mod(nc, bcpool, modAR, st, 3, False, "bcB")
            h2 = work.tile([128, 1024], F32, tag="h1")
            nc.vector.tensor_mul(h2[:], ln[:], A2[:])
            nc.vector.tensor_add(h2[:], h2[:], B2[:])
            nc.sync.dma_start(h2_out.ap()[st * 128:(st + 1) * 128, :], h2[:])
            _transpose_to(nc, psum, ident, h2, h2T, st)
        gw_sb = const.tile([128, 8, 8], F32, tag="gatew")
        nc.sync.dma_start(gw_sb[:],
                          gate_w_in.ap().rearrange("(o p) e -> p o e", p=128))
        lg_ps = psum.tile([8, 256], F32, tag="ps512", name="lg_ps")
        for k in range(8):
            nc.tensor.matmul(lg_ps[:], gw_sb[:, k, :], h2T[:, k, :],
                             start=(k == 0), stop=(k == 7))
        lg_sb = work.tile([8, 256], F32, tag="lg_sb", bufs=1)
        nc.vector.tensor_copy(lg_sb[:], lg_ps[:])
        nc.sync.dma_start(logitsT_out.ap(), lg_sb[:])

    nc.compile()
    return nc


# ---------------------------------------------------------------- launch 2
def build_launch2():
    nc = bacc.Bacc("TRN2", target_bir_lowering=False, debug=False,
                   num_devices=NC)
    xgT = nc.dram_tensor("xgT", [1024, CAP], BF16, kind="ExternalInput")
    w1 = nc.dram_tensor("w1", [1024, 2048], BF16, kind="ExternalInput")
    b1T = nc.dram_tensor("b1T", [128, 16], F32, kind="ExternalInput")
    w2 = nc.dram_tensor("w2", [2048, 1024], BF16, kind="ExternalInput")
    b2T = nc.dram_tensor("b2T", [128, 8], F32, kind="ExternalInput")
    yT_out = nc.dram_tensor("yT_out", [1024, CAP], F32, kind="ExternalOutput")

    NB = [(0, 512), (512, CAP - 512)] if CAP > 512 else [(0, CAP)]
    with tile.TileContext(nc) as tc, ExitStack() as ctx:
        big = ctx.enter_context(tc.tile_pool(name="big", bufs=1))
        work = ctx.enter_context(tc.tile_pool(name="work", bufs=3))
        psum = ctx.enter_context(tc.tile_pool(name="ps", bufs=2, space="PSUM"))

        xg_sb = big.tile([128, 8, CAP], BF16, tag="xg")
        nc.sync.dma_start(xg_sb[:], xgT.ap().rearrange("(o p) t -> p o t", p=128))
        w1_sb = big.tile([128, 8, 2048], BF16, tag="w1")
        nc.sync.dma_start(w1_sb[:], w1.ap().rearrange("(o p) m -> p o m", p=128))
        w2_sb = big.tile([128, 16, 1024], BF16, tag="w2")
        nc.sync.dma_start(w2_sb[:], w2.ap().rearrange("(o p) m -> p o m", p=128))
        b1_sb = work.tile([128, 16], F32, tag="b1", bufs=1)
        nc.sync.dma_start(b1_sb[:], b1T.ap())
        b2_sb = work.tile([128, 8], F32, tag="b2", bufs=1)
        nc.sync.dma_start(b2_sb[:], b2T.ap())
        act = big.tile([128, 16, CAP], BF16, tag="act")

        for m in range(16):
            for (n0, nn) in NB:
                pp = psum.tile([128, 512], F32, tag="ps1", name=f"ps1_{m}_{n0}")
                for k in range(8):
                    nc.tensor.matmul(
                        pp[:, :nn], w1_sb[:, k, m * 128:(m + 1) * 128],
                        xg_sb[:, k, n0:n0 + nn],
                        start=(k == 0), stop=(k == 7))
                nc.scalar.activation(
                    out=act[:, m, n0:n0 + nn], in_=pp[:, :nn], func=AF.Gelu,
                    bias=b1_sb[:, m:m + 1])
        yT_r = yT_out.ap().rearrange("(o p) t -> p o t", p=128)
        for m in range(8):
            for (n0, nn) in NB:
                pp = psum.tile([128, 512], F32, tag="ps2", name=f"ps2_{m}_{n0}")
                for k in range(16):
                    nc.tensor.matmul(
                        pp[:, :nn], w2_sb[:, k, m * 128:(m + 1) * 128],
                        act[:, k, n0:n0 + nn],
                        start=(k == 0), stop=(k == 15))
                y_sb = work.tile([128, 512], F32, tag="y_sb")
                nc.scalar.activation(
                    out=y_sb[:, :nn], in_=pp[:, :nn], func=AF.Identity,
                    bias=b2_sb[:, m:m + 1])
                nc.sync.dma_start(yT_r[:, m, n0:n0 + nn], y_sb[:, :nn])
    nc.compile()
    return nc


# ---------------------------------------------------------------- host glue
def _get_nc(which):
    with _lock:
        if which not in _cache:
            _cache[which] = build_launch1() if which == 1 else build_launch2()
        return _cache[which]


def _bf(a):
    import ml_dtypes
    return np.ascontiguousarray(np.asarray(a, np.float32)).astype(
        ml_dtypes.bfloat16)


def _prep_launch1_inputs(x, condition, enc, rope_cos, rope_sin, adaln_w,
                         adaln_b, qw, qb, kw, kb, vw, vb, ow, ob, cqw, cqb,
                         ckw, ckb, cvw, cvb, cow, cob, cn_g, cn_b, gate_w):
    f32 = np.float32
    x = np.asarray(x, f32)
    sc = f32(1.0 / np.sqrt(HD))
    condT = np.ascontiguousarray(np.asarray(condition, f32).T)      # [1024,2]
    encT = _bf(np.asarray(enc, f32).reshape(B * L, C).T)            # [2048,512]
    cosT = np.asarray(rope_cos, f32).T                              # [64,1024]
    sinT = np.asarray(rope_sin, f32).T
    ct2 = np.tile(cosT, (2, 1))      # [128, 1024]: rows = 2 heads x 64 dims
    st2 = np.tile(sinT, (2, 1))
    cosQ = np.empty((128, 8, 2, 128), f32)
    sinQ = np.empty((128, 8, 2, 128), f32)
    for c_ in range(8):
        for b_ in range(2):
            cosQ[:, c_, b_, :] = ct2[:, c_ * 128:(c_ + 1) * 128]
            sinQ[:, c_, b_, :] = st2[:, c_ * 128:(c_ + 1) * 128]
    cosQ = _bf(cosQ.reshape(128, 2048))
    sinQ = _bf(sinQ.reshape(128, 2048))
    # rot_half permutation: out[m] = -q[m+32] (m%64<32), +q[m-32] (else)
    perm = np.zeros((128, 128), f32)
    for h2 in range(2):
        for d in range(32):
            perm[h2 * 64 + d + 32, h2 * 64 + d] = -1.0
            perm[h2 * 64 + d, h2 * 64 + d + 32] = 1.0
    ob_bc = np.tile(np.asarray(ob, f32)[None, :], (128, 1))
    cob_bc = np.tile(np.asarray(cob, f32)[None, :], (128, 1))
    cng_bc = np.tile(np.asarray(cn_g, f32)[None, :], (128, 1))
    cnb_bc = np.tile(np.asarray(cn_b, f32)[None, :], (128, 1))
    adaln_b8 = np.ascontiguousarray(
        np.tile(np.asarray(adaln_b, f32)[None, :] / NC, (2, 1)))
    qw8 = np.asarray(qw, f32) * sc
    qb8 = np.asarray(qb, f32) * sc
    cqw8 = np.asarray(cqw, f32) * sc
    cqb8 = np.asarray(cqb, f32) * sc

    in_maps = []
    for c in range(NC):
        sl = slice(128 * c, 128 * (c + 1))
        xch = np.concatenate([x[0, sl], x[1, sl]], axis=0)
        in_maps.append(dict(
            xch=np.ascontiguousarray(xch),
            condTs=np.ascontiguousarray(condT[sl]),
            adaln_ws=_bf(np.asarray(adaln_w, f32)[sl]),
            adaln_b8=adaln_b8,
            qwh=_bf(qw8[:, sl]),
            qbh=np.ascontiguousarray(qb8[sl])[:, None],
            kwh=_bf(np.asarray(kw, f32)[:, sl]),
            kbh=np.ascontiguousarray(np.asarray(kb, f32)[sl])[:, None],
            vwh=_bf(np.asarray(vw, f32)[:, sl]),
            vbh_bc=np.ascontiguousarray(
                np.tile(np.asarray(vb, f32)[None, sl], (128, 1))),
            cosQ=cosQ, sinQ=sinQ, perm128=_bf(perm),
            ow_in=_bf(ow), ob_bc=ob_bc,
            encT=encT,
            cqwh=_bf(cqw8[:, sl]),
            cqbh=np.ascontiguousarray(cqb8[sl])[:, None],
            ckwh=_bf(np.asarray(ckw, f32)[:, sl]),
            ckbh=np.ascontiguousarray(np.asarray(ckb, f32)[sl])[:, None],
            cvwh=_bf(np.asarray(cvw, f32)[:, sl]),
            cvbh_bc=np.ascontiguousarray(
                np.tile(np.asarray(cvb, f32)[None, sl], (128, 1))),
            cow_in=_bf(cow), cob_bc=cob_bc,
            cng_bc=cng_bc, cnb_bc=cnb_bc,
            gate_w_in=np.ascontiguousarray(np.asarray(gate_w, f32)),
        ))
    return in_maps


def kernel(x, condition, enc, rope_cos, rope_sin, adaln_w, adaln_b, qw, qb,
           kw, kb, vw, vb, ow, ob, cqw, cqb, ckw, ckb, cvw, cvb, cow, cob,
           cn_g, cn_b, gate_w, ew1, eb1, ew2, eb2):
    f32 = np.float32
    trace = bool(int(os.environ.get("KERNEL_TRACE", "0")))
    if trace:
        import trace_shim
        trace_shim.install()

    in_maps = _prep_launch1_inputs(
        x, condition, enc, rope_cos, rope_sin, adaln_w, adaln_b, qw, qb, kw,
        kb, vw, vb, ow, ob, cqw, cqb, ckw, ckb, cvw, cvb, cow, cob, cn_g,
        cn_b, gate_w)
    nc1 = _get_nc(1)
    res1 = run_bass_kernel_spmd(nc1, in_maps, core_ids=list(range(NC)),
                                trace=trace)
    kernel.last_exec1 = res1.exec_time_ns

    x2 = np.empty((B, S, H), f32)
    h2 = np.empty((B, S, H), f32)
    logits = np.empty((B, S, E), f32)
    for c in range(NC):
        r = res1.results[c]
        sl = slice(128 * c, 128 * (c + 1))
        x2[0, sl] = r["x2_out"][:128]
        x2[1, sl] = r["x2_out"][128:]
        h2[0, sl] = r["h2_out"][:128]
        h2[1, sl] = r["h2_out"][128:]
        lg = r["logitsT_out"].T
        logits[0, sl] = lg[:128]
        logits[1, sl] = lg[128:]
    mod = res1.results[0]["mod_out"]
    g2 = mod[:, 5 * H:6 * H]

    # ---- routing (host, fp32)
    lg2 = logits.reshape(T, E)
    p = np.exp(lg2 - lg2.max(-1, keepdims=True))
    probs = p / p.sum(-1, keepdims=True)
    order = np.argsort(-probs, axis=-1, kind="stable")
    topi = order[:, :TOPK]
    topp = np.take_along_axis(probs, topi, axis=-1)
    topp = topp / topp.sum(-1, keepdims=True)
    cw = np.zeros((T, E), f32)
    np.put_along_axis(cw, topi, topp.astype(f32), axis=-1)

    onehot_sum = np.zeros((B, S, E), f32)
    ti = topi.reshape(B, S, TOPK)
    for kk in range(TOPK):
        for b_ in range(B):
            onehot_sum[b_, np.arange(S), ti[b_, :, kk]] += 1.0
    tokens_per_expert = onehot_sum.mean(0)
    avg_prob = probs.reshape(B, S, E).mean((0, 1))
    aux = np.float32(E * (tokens_per_expert * avg_prob).sum())

    # ---- expert-parallel FFN (launch 2)
    h2f = h2.reshape(T, H)
    tok_lists = [np.where(cw[:, e] > 0)[0] for e in range(E)]
    n_batches = max(1, max((len(t) + CAP - 1) // CAP for t in tok_lists))
    nc2 = _get_nc(2)
    ew1 = np.asarray(ew1, f32)
    ew2 = np.asarray(ew2, f32)
    eb1 = np.asarray(eb1, f32)
    eb2 = np.asarray(eb2, f32)
    w1s = [_bf(ew1[e]) for e in range(E)]
    w2s = [_bf(ew2[e]) for e in range(E)]
    b1Ts = [np.ascontiguousarray(eb1[e].reshape(16, 128).T) for e in range(E)]
    b2Ts = [np.ascontiguousarray(eb2[e].reshape(8, 128).T) for e in range(E)]
    moe = np.zeros((T, H), f32)
    kernel.last_exec2 = 0
    for bi in range(n_batches):
        maps2 = []
        batch_toks = []
        for e in range(E):
            toks = tok_lists[e][bi * CAP:(bi + 1) * CAP]
            batch_toks.append(toks)
            xg = np.zeros((CAP, H), f32)
            xg[:len(toks)] = h2f[toks]
            maps2.append(dict(xgT=_bf(xg.T), w1=w1s[e], b1T=b1Ts[e],
                              w2=w2s[e], b2T=b2Ts[e]))
        res2 = run_bass_kernel_spmd(nc2, maps2, core_ids=list(range(NC)),
                                    trace=trace)
        if trace and res2.exec_time_ns:
            kernel.last_exec2 += res2.exec_time_ns
        for e in range(E):
            toks = batch_toks[e]
            if len(toks) == 0:
                continue
            y = res2.results[e]["yT_out"].T[:len(toks)]
            moe[toks] += cw[toks, e][:, None] * y

    out = x2 + g2[:, None, :] * moe.reshape(B, S, H)
    return out.astype(f32), aux


# revision 13
# speedup vs baseline: 1.1476x; 1.1476x over previous
"""Trainium2 Bass kernel for nn_BagleyDiT (DiT block: AdaLN + self-attn with
RoPE + cross-attn + top-2 MoE FFN), SPMD across 8 NeuronCores.

Sharding:
  - tokens: core c owns positions [128c, 128c+128) of BOTH batches (256 rows)
  - attention: head-parallel (2 of 16 heads per core); activations exchanged
    via AllGather in feature-major bf16
  - AdaLN modulation: contraction-sharded matmul + AllReduce; the per-batch
    LN affine (1+sc)*ln + sh is folded into the QKV weights/biases on device
    so the AllReduce overlaps the ln AllGather (cn_g/cn_b are folded on host)
  - MoE: expert-parallel second launch (core c = expert c), host routes.

Activation layout conventions:
  token-major  [tokens(part), feat(free)] for LN / residual / pointwise
  feature-major[feat(part), tokens(free)] for matmul operands; global token
  column order is (chunk c', batch b, pos j) -> col 256c' + 128b + j.

SBUF/PSUM pools are statically allocated per tag, so tags are deliberately
aliased across phases with disjoint lifetimes.
"""

import os
import sys
import threading
from contextlib import ExitStack

import numpy as np

sys.path.insert(0, "/opt/trn_rl_repo")

import concourse.bass as bass  # noqa: E402
import concourse.mybir as mybir  # noqa: E402
import concourse.tile as tile  # noqa: E402
from concourse import bacc  # noqa: E402
from concourse.bass import ds  # noqa: E402
from concourse.bass_utils import run_bass_kernel_spmd  # noqa: E402
from concourse.masks import make_identity  # noqa: E402

B, S, H, NH, HD = 2, 1024, 1024, 16, 64
L, C, E, TOPK, D = 256, 2048, 8, 2, 2048
EPS = 1e-5
NC = 8
T = B * S
CAP = 768       # launch-2 tokens per expert per batch

F32 = mybir.dt.float32
BF16 = mybir.dt.bfloat16
AF = mybir.ActivationFunctionType
ALU = mybir.AluOpType

RG = [list(range(NC))]

_cache = {}
_lock = threading.Lock()


# ---------------------------------------------------------------- helpers
def _ln_tok(nc, pool, x_t, eps_t):
    """LayerNorm over free dim of token-major x_t [128, 1024] f32."""
    st = pool.tile([128, 2, 6], F32, tag="ln_st")
    mv = pool.tile([128, 2], F32, tag="ln_mv")
    nc.vector.bn_stats(out=st[:, 0, :], in_=x_t[:, 0:512])
    nc.vector.bn_stats(out=st[:, 1, :], in_=x_t[:, 512:1024])
    nc.vector.bn_aggr(out=mv[:], in_=st[:])
    rstd = pool.tile([128, 1], F32, tag="ln_rstd")
    nc.scalar.activation(out=rstd[:], in_=mv[:, 1:2], func=AF.Sqrt, bias=eps_t[:])
    nc.vector.reciprocal(out=rstd[:], in_=rstd[:])
    ln = pool.tile([128, 1024], F32, tag="ln_out")
    nc.vector.tensor_scalar(
        out=ln[:], in0=x_t[:], scalar1=mv[:, 0:1], scalar2=rstd[:],
        op0=ALU.subtract, op1=ALU.mult)
    return ln


def _transpose_to(nc, pspool, ident, src_t, dst_t, st):
    """PE-transpose token-major src_t [128, 1024] into feature-major dst_t
    [128, 8, 256] at [:, o, 128*st : +128] (casts to dst dtype on evict)."""
    for o in range(8):
        pt = pspool.tile([128, 128], F32, tag="psmisc", name=f"tr_ps{o}")
        nc.tensor.transpose(pt[:], src_t[:, o * 128:(o + 1) * 128], ident[:])
        nc.scalar.activation(
            out=dst_t[:, o, st * 128:(st + 1) * 128], in_=pt[:], func=AF.Copy)


def _bcast_from_mod(nc, pool, mod_dram, b, seg, add_one, tag):
    """DMA-broadcast mod[b, 1024*seg : +1024] to all partitions [128,1024]."""
    t = pool.tile([128, 1024], F32, tag=tag, name=f"bc_{tag}_{b}_{seg}")
    src = mod_dram[:]
    ap = bass.AP(
        tensor=src.tensor,
        offset=src.offset + b * 6 * H + seg * H,
        ap=[[0, 128], [1, 1024]],
    )
    nc.sync.dma_start(out=t[:], in_=ap)
    if add_one:
        nc.vector.tensor_scalar_add(t[:], t[:], 1.0)
    return t


def _mod_colvec(nc, pool, mod_dram, b, seg, add_one, name):
    """Load mod[b, 1024*seg : +1024] feature-major as [128, 8] (f = 128o+p)."""
    t = pool.tile([128, 8], F32, tag="modcol", name=name)
    src = mod_dram[:]
    ap = bass.AP(
        tensor=src.tensor,
        offset=src.offset + b * 6 * H + seg * H,
        ap=[[1, 128], [128, 8]],
    )
    nc.sync.dma_start(out=t[:], in_=ap)
    if add_one:
        nc.vector.tensor_scalar_add(t[:], t[:], 1.0)
    return t


def _attention(nc, work, psum, qT, kT, v_sb, out_b, n_tkblocks, tkb_of,
               k_is_flat=False):
    """Head-sharded attention for 2 local heads x 2 batches.

    qT [128, 2048] bf16, cols (c' 8, b 2, j 128). kT: same layout (self) or
    [128, 512] cols (b*256 + l) (cross). v_sb [128, ntb, 2, 65] token-major
    with a ones column at 64 (softmax row-sums come out as av row 64).
    out_b: two tiles [128, 1024] bf16, cols (c' 8, j 128), one per batch.
    Softmax = plain exp (scores bounded by construction; no max-sub)."""
    q4 = qT.rearrange("p (c s j) -> p c s j", c=8, s=2)
    for b in range(2):
        o3 = out_b[b].rearrange("p (c j) -> p c j", c=8)
        for hh in range(2):
            hsl = slice(hh * 64, hh * 64 + 64)
            av = [psum.tile([128, 512], F32, tag=f"av{nt}", bufs=2,
                            name=f"av{nt}_{b}_{hh}")
                  for nt in range(2)]
            for mt in range(n_tkblocks):
                if k_is_flat:
                    lhsK = kT[hsl, b * 256 + mt * 128: b * 256 + (mt + 1) * 128]
                else:
                    lhsK = kT.rearrange(
                        "p (c s j) -> p c s j", c=8, s=2)[hsl, mt, b, :]
                for nt in range(2):
                    sp = psum.tile([128, 512], F32, tag="ps512",
                                   name=f"s_ps{nt}")
                    nc.tensor.matmul(
                        sp[:], lhsK, q4[hsl, 4 * nt:4 * nt + 4, b, :],
                        start=True, stop=True)
                    p_sb = work.tile([128, 512], BF16, tag="p_sb", bufs=3)
                    nc.scalar.activation(out=p_sb[:], in_=sp[:], func=AF.Exp)
                    nc.tensor.matmul(
                        av[nt][0:65, :], v_sb[:, tkb_of(b, mt), hh, :],
                        p_sb[:], start=(mt == 0), stop=(mt == n_tkblocks - 1))
            rec = work.tile([1, 1024], F32, tag="rec", bufs=1,
                            name=f"rec_{b}_{hh}")
            for nt in range(2):
                nc.vector.reciprocal(
                    out=rec[:, nt * 512:(nt + 1) * 512], in_=av[nt][64:65, :])
            rb = work.tile([64, 1024], F32, tag="rb", bufs=1,
                           name=f"rb_{b}_{hh}")
            nc.gpsimd.partition_broadcast(rb[:], rec[:])
            for nt in range(2):
                nc.vector.tensor_tensor(
                    o3[hsl, 4 * nt:4 * nt + 4, :],
                    av[nt][0:64, :].rearrange("p (c j) -> p c j", c=4),
                    rb[:, nt * 512:(nt + 1) * 512].rearrange(
                        "p (c j) -> p c j", c=4),
                    ALU.mult)


def _proj_residual(nc, work, psum, big, bcpool, pid, ag_b, w_sb, b_bc, xin_t,
                   gate, tag):
    """out = xin + [gate_b *] (own_chunk(ag_b[st]) @ W + bias), token-major.
    ag_b: per-batch DRAM [1024, 1024] bf16 (rows=feat, cols=(c',j)); own
    chunk cols = [128*pid : +128]. Returns 2 tiles [128, 1024] f32."""
    bias_sb = work.tile([128, 1024], F32, tag="bias_pr", bufs=2,
                        name=f"bias_{tag}")
    nc.sync.dma_start(bias_sb[:], b_bc.ap())
    out_t = [None, None]
    for st in range(2):
        aT_own = big.tile([128, 8, 128], BF16, tag="aown", bufs=2,
                          name=f"aown_{tag}_{st}")
        nc.sync.dma_start(
            aT_own[:],
            ag_b[st][:].rearrange("(k p) t -> p k t", p=128)
            [:, :, ds(pid * 128, 128)])
        out_t[st] = big.tile([128, 1024], F32, tag=f"acc_{tag}_{st}", bufs=1,
                             name=f"out_{tag}_{st}")
        g_bc = None
        if gate is not None:
            g_bc = _bcast_from_mod(nc, bcpool, gate[0], st, gate[1], False,
                                   "bcA")
        for nt in range(2):
            pp = psum.tile([128, 512], F32, tag="ps512", name=f"proj_ps{nt}")
            for k in range(8):
                nc.tensor.matmul(
                    pp[:], aT_own[:, k, :],
                    w_sb[:, k, nt * 512:(nt + 1) * 512],
                    start=(k == 0), stop=(k == 7))
            sl = slice(nt * 512, (nt + 1) * 512)
            tmp = work.tile([128, 512], F32, tag="prj_tmp")
            nc.vector.tensor_add(tmp[:], pp[:], bias_sb[:, sl])
            if g_bc is not None:
                nc.vector.tensor_mul(tmp[:], tmp[:], g_bc[:, sl])
            nc.vector.tensor_add(out_t[st][:, sl], tmp[:], xin_t[st][:, sl])
    return out_t


# ---------------------------------------------------------------- launch 1
def build_launch1():
    nc = bacc.Bacc("TRN2", target_bir_lowering=False, debug=False,
                   num_devices=NC)

    def din(name, shape, dt=F32):
        return nc.dram_tensor(name, shape, dt, kind="ExternalInput")

    xch = din("xch", [256, 1024])
    condTs = din("condTs", [128, 2])
    adaln_ws = din("adaln_ws", [128, 6144], BF16)
    adaln_b8 = din("adaln_b8", [2, 6144])
    qwh = din("qwh", [1024, 128], BF16)
    kwh = din("kwh", [1024, 128], BF16)
    vwh = din("vwh", [1024, 128], BF16)
    qbh = din("qbh", [128, 1])
    kbh = din("kbh", [128, 1])
    vbh_bc = din("vbh_bc", [128, 128])
    cosQ = din("cosQ", [128, 2048], BF16)
    sinQ = din("sinQ", [128, 2048], BF16)
    perm128 = din("perm128", [128, 128], BF16)
    ow_in = din("ow_in", [1024, 1024], BF16)
    ob_bc = din("ob_bc", [128, 1024])
    encT = din("encT", [2048, 512], BF16)
    cqwh = din("cqwh", [1024, 128], BF16)     # cn_g pre-folded on host
    cqbh = din("cqbh", [128, 1])              # cn_b pre-folded on host
    ckwh = din("ckwh", [2048, 128], BF16)
    ckbh = din("ckbh", [128, 1])
    cvwh = din("cvwh", [2048, 128], BF16)
    cvbh_bc = din("cvbh_bc", [128, 128])
    cow_in = din("cow_in", [1024, 1024], BF16)
    cob_bc = din("cob_bc", [128, 1024])
    gate_w_in = din("gate_w_in", [1024, 8])

    x2_out = nc.dram_tensor("x2_out", [256, 1024], F32, kind="ExternalOutput")
    h2_out = nc.dram_tensor("h2_out", [256, 1024], F32, kind="ExternalOutput")
    logitsT_out = nc.dram_tensor("logitsT_out", [8, 256], F32,
                                 kind="ExternalOutput")
    mod_out = nc.dram_tensor("mod_out", [2, 6144], F32, kind="ExternalOutput")

    with tile.TileContext(nc) as tc, ExitStack() as ctx:
        const = ctx.enter_context(tc.tile_pool(name="const", bufs=1))
        big = ctx.enter_context(tc.tile_pool(name="big", bufs=1))
        work = ctx.enter_context(tc.tile_pool(name="work", bufs=2))
        bcpool = ctx.enter_context(tc.tile_pool(name="bcast", bufs=1))
        psum = ctx.enter_context(tc.tile_pool(name="psum", bufs=2, space="PSUM"))
        dram = ctx.enter_context(tc.tile_pool(name="dram", bufs=1, space="DRAM"))

        ident = const.tile([128, 128], F32)
        make_identity(nc, ident)
        identbf = const.tile([128, 128], BF16)
        make_identity(nc, identbf)
        eps_t = const.tile([128, 1], F32)
        nc.vector.memset(eps_t, EPS)

        pid = nc.sync.partition_id()

        # -------- Phase 0a: AdaLN modulation matmul + AllReduce (async wrt
        # everything below until the eff-weight build).
        cond_sb = work.tile([128, 2], F32, tag="cond")
        nc.sync.dma_start(cond_sb[:], condTs.ap())
        silu_bf = work.tile([128, 2], BF16, tag="silu")
        nc.scalar.activation(out=silu_bf[:], in_=cond_sb[:], func=AF.Silu)
        modAR_in = dram.tile([2, 6144], F32)
        modAR = dram.tile([2, 6144], F32, addr_space="Shared")
        for j in range(12):
            jsl = slice(j * 512, (j + 1) * 512)
            aw_j = work.tile([128, 512], BF16, tag="aw_j", bufs=3,
                             name=f"aw_{j}")
            nc.sync.dma_start(aw_j[:], adaln_ws.ap()[:, jsl])
            bp = work.tile([2, 512], F32, tag="biasp", name=f"bp_{j}")
            nc.sync.dma_start(bp[:], adaln_b8.ap()[:, jsl])
            mp = psum.tile([2, 512], F32, tag="ps512", name=f"mod_ps{j}")
            nc.tensor.matmul(mp[:], silu_bf[:], aw_j[:],
                             start=True, stop=True)
            mo = work.tile([2, 512], F32, tag="modp", name=f"mo_{j}")
            nc.vector.tensor_add(out=mo[:], in0=mp[:], in1=bp[:])
            nc.sync.dma_start(modAR_in[:, jsl], mo[:])
        nc.gpsimd.collective_compute(
            "AllReduce", ALU.add, replica_groups=RG,
            ins=[modAR_in.opt()], outs=[modAR.opt()])
        nc.sync.dma_start(mod_out.ap(), modAR[:])

        # -------- Phase 1: ln(x) -> transpose -> AllGather (no mod dep!)
        x_t = [None, None]
        for st in range(2):
            x_t[st] = big.tile([128, 1024], F32, tag=f"x_{st}", bufs=1,
                               name=f"x_{st}")
            nc.sync.dma_start(x_t[st][:], xch.ap()[st * 128:(st + 1) * 128, :])

        hT = work.tile([128, 8, 256], BF16, tag="hT", bufs=1)
        for st in range(2):
            ln = _ln_tok(nc, work, x_t[st], eps_t)
            _transpose_to(nc, psum, ident, ln, hT, st)
        hAG_in = dram.tile([1024, 256], BF16)
        hAG = dram.tile([8192, 256], BF16, addr_space="Shared")
        nc.sync.dma_start(hAG_in[:].rearrange("(o p) t -> p o t", p=128), hT[:])
        nc.gpsimd.collective_compute(
            "AllGather", ALU.bypass, replica_groups=RG,
            ins=[hAG_in.opt()], outs=[hAG.opt()])

        # -------- Phase 0b (early): cross-attn K/V from enc + weight prefetch
        ckw_sb = const.tile([128, 16, 128], BF16, tag="ckw")
        cvw_sb = const.tile([128, 16, 128], BF16, tag="cvw")
        nc.sync.dma_start(ckw_sb[:],
                          ckwh.ap().rearrange("(o p) m -> p o m", p=128))
        nc.sync.dma_start(cvw_sb[:],
                          cvwh.ap().rearrange("(o p) m -> p o m", p=128))
        ckb_sb = const.tile([128, 1], F32, tag="ckb")
        nc.sync.dma_start(ckb_sb[:], ckbh.ap())
        cvb_sb = const.tile([128, 128], F32, tag="cvb")
        nc.sync.dma_start(cvb_sb[:], cvbh_bc.ap())
        lk = work.tile([128, 512], BF16, tag="lk", bufs=1)
        lv = big.tile([128, 4, 2, 65], BF16, tag="lv", bufs=1)
        nc.vector.memset(lv[:, :, :, 64:65], 1.0)
        lkp = psum.tile([128, 512], F32, tag="av0", bufs=2, name="lk_ps")
        lv_tags = [("av1", 2), ("psmisc", 2), ("psmisc", 2), ("ps512", 2)]
        lvp = [psum.tile([128, 128], F32, tag=lv_tags[tb][0],
                         bufs=lv_tags[tb][1], name=f"lv_ps{tb}")
               for tb in range(4)]
        encT_r = encT.ap().rearrange("(o p) n -> p o n", p=128)
        for k in range(16):
            et = work.tile([128, 512], BF16, tag="encT_k", bufs=3,
                           name=f"encT_k{k}")
            nc.sync.dma_start(et[:], encT_r[:, k, :])
            nc.tensor.matmul(lkp[:], ckw_sb[:, k, :], et[:],
                             start=(k == 0), stop=(k == 15))
            for tb in range(4):
                nc.tensor.matmul(
                    lvp[tb][:], et[:, tb * 128:(tb + 1) * 128],
                    cvw_sb[:, k, :], start=(k == 0), stop=(k == 15))
        nc.scalar.activation(out=lk[:], in_=lkp[:], func=AF.Identity,
                             bias=ckb_sb[:])
        for tb in range(4):
            for hh in range(2):
                nc.vector.tensor_add(
                    out=lv[:, tb, hh, 0:64],
                    in0=lvp[tb][:, hh * 64:hh * 64 + 64],
                    in1=cvb_sb[:, hh * 64:hh * 64 + 64])

        # prefetch big projection weights early
        ow_sb = big.tile([128, 8, 1024], BF16, tag="wbig", bufs=1, name="w_ow")
        nc.sync.dma_start(ow_sb[:],
                          ow_in.ap().rearrange("(o p) m -> p o m", p=128))
        cow_sb = big.tile([128, 8, 1024], BF16, tag="wbig", bufs=1,
                          name="w_cow")
        nc.sync.dma_start(cow_sb[:],
                          cow_in.ap().rearrange("(o p) m -> p o m", p=128))

        # -------- Phase 2a: effective QKV weights (fold LN affine, per batch)
        qw_sb = const.tile([128, 8, 128], BF16, tag="qw")
        kw_sb = const.tile([128, 8, 128], BF16, tag="kw")
        vw_sb = const.tile([128, 8, 128], BF16, tag="vw")
        nc.sync.dma_start(qw_sb[:], qwh.ap().rearrange("(o p) m -> p o m", p=128))
        nc.sync.dma_start(kw_sb[:], kwh.ap().rearrange("(o p) m -> p o m", p=128))
        nc.sync.dma_start(vw_sb[:], vwh.ap().rearrange("(o p) m -> p o m", p=128))
        qb_sb = const.tile([128, 1], F32, tag="qb")
        kb_sb = const.tile([128, 1], F32, tag="kb")
        vb_sb = const.tile([128, 128], F32, tag="vb")
        nc.sync.dma_start(qb_sb[:], qbh.ap())
        nc.sync.dma_start(kb_sb[:], kbh.ap())
        nc.sync.dma_start(vb_sb[:], vbh_bc.ap())

        # A1/B1 per batch in feature-major column form [128, 8]
        weff = {}
        for b in range(2):
            A1 = _mod_colvec(nc, work, modAR, b, 1, True, f"A1_{b}")
            B1 = _mod_colvec(nc, work, modAR, b, 0, False, f"B1_{b}")
            B1bf = work.tile([128, 8], BF16, tag="modcolbf",
                             name=f"B1bf_{b}")
            nc.vector.tensor_copy(B1bf[:], B1[:])
            for nm, w0, b0 in (("q", qw_sb, qb_sb), ("k", kw_sb, kb_sb),
                               ("v", vw_sb, None)):
                we = big.tile([128, 8, 128], BF16, tag=f"weff_{nm}{b}", bufs=1,
                              name=f"weff_{nm}{b}")
                nc.vector.tensor_tensor(
                    we[:], w0[:], A1[:, :, None].to_broadcast((128, 8, 128)),
                    ALU.mult)
                bp = psum.tile([128, 128], F32, tag="psmisc",
                               name=f"beff_ps_{nm}{b}")
                for k in range(8):
                    nc.tensor.matmul(bp[:, 0:1], w0[:, k, :], B1bf[:, k:k + 1],
                                     start=(k == 0), stop=(k == 7))
                if nm == "v":
                    # v bias is per-free(dim): transpose column -> row, bcast
                    bcol = work.tile([128, 1], BF16, tag="vbcol",
                                     name=f"vbcol_{b}")
                    nc.vector.tensor_copy(bcol[:], bp[:, 0:1])
                    brow = psum.tile([128, 128], BF16, tag="psmisc",
                                     name=f"brow_{b}")
                    nc.tensor.transpose(brow[0:1, :], bcol[:], identbf[:])
                    br_sb = work.tile([1, 128], F32, tag="vbrow",
                                      name=f"vbrow_{b}")
                    nc.vector.tensor_copy(br_sb[:], brow[0:1, :])
                    be = big.tile([128, 128], F32, tag=f"beff_v{b}", bufs=1,
                                  name=f"beff_v{b}")
                    nc.gpsimd.partition_broadcast(be[:], br_sb[:])
                    nc.vector.tensor_add(be[:], be[:], vb_sb[:])
                else:
                    be = work.tile([128, 1], F32, tag=f"beff_{nm}{b}", bufs=1,
                                   name=f"beff_{nm}{b}")
                    nc.vector.tensor_add(be[:], bp[:, 0:1], b0[:])
                weff[(nm, b)] = (we, be)

        # -------- Phase 2b: QKV projections (+RoPE on q, k)
        q_raw = big.tile([128, 2048], BF16, tag="t2048a", bufs=1, name="q_raw")
        k_raw = big.tile([128, 2048], BF16, tag="t2048b", bufs=1, name="k_raw")
        v_sb = big.tile([128, 16, 2, 65], BF16, tag="v_sb", bufs=1)
        nc.vector.memset(v_sb[:, :, :, 64:65], 1.0)

        hAG_r = hAG[:].rearrange("(c k p) t -> p k c t", p=128, k=8)
        q4w = q_raw.rearrange("p (c s j) -> p c s j", c=8, s=2)
        k4w = k_raw.rearrange("p (c s j) -> p c s j", c=8, s=2)

        def qkv_pass(src_r, wq_b, bq_b, dst4, wk_b=None, bk_b=None,
                     dst4k=None, make_v=False):
            """dst4[:, c, b, :] = W_eff_b.T @ lnT cols; optional k and v."""
            for half in range(2):
                csl = slice(4 * half, 4 * half + 4)
                hk = []
                for k in range(8):
                    t = big.tile([128, 4, 256], BF16, tag=f"hT_k{k}", bufs=1,
                                 name=f"hT_k{k}_{half}")
                    nc.sync.dma_start(t[:], src_r[:, k, csl, :])
                    hk.append(t)
                for b in range(2):
                    rhs_sl = lambda k: hk[k].rearrange(
                        "p c (s j) -> p c s j", s=2)[:, :, b, :]
                    qp = psum.tile([128, 512], F32, tag="ps512",
                                   name=f"q_ps{half}{b}")
                    kp = None
                    if wk_b is not None:
                        kp = psum.tile([128, 512], F32, tag="ps512",
                                       name=f"k_ps{half}{b}")
                    for k in range(8):
                        rhs = rhs_sl(k)
                        nc.tensor.matmul(qp[:], wq_b[b][:, k, :], rhs,
                                         start=(k == 0), stop=(k == 7))
                        if kp is not None:
                            nc.tensor.matmul(kp[:], wk_b[b][:, k, :], rhs,
                                             start=(k == 0), stop=(k == 7))
                    nc.scalar.activation(
                        out=dst4[:, 4 * half:4 * half + 4, b, :],
                        in_=qp[:].rearrange("p (c j) -> p c j", c=4),
                        func=AF.Identity, bias=bq_b[b][:])
                    if kp is not None:
                        nc.scalar.activation(
                            out=dst4k[:, 4 * half:4 * half + 4, b, :],
                            in_=kp[:].rearrange("p (c j) -> p c j", c=4),
                            func=AF.Identity, bias=bk_b[b][:])
                if make_v:
                    for tb2 in range(8):
                        c2, st_ = tb2 // 2, tb2 % 2
                        tb = 8 * half + tb2
                        vp = psum.tile([128, 128], F32, tag="psmisc",
                                       name=f"v_ps{tb}")
                        for k in range(8):
                            nc.tensor.matmul(
                                vp[:], hk[k][:, c2, st_ * 128:(st_ + 1) * 128],
                                weff[("v", st_)][0][:, k, :],
                                start=(k == 0), stop=(k == 7))
                        for hh in range(2):
                            nc.vector.tensor_add(
                                out=v_sb[:, tb, hh, 0:64],
                                in0=vp[:, hh * 64:hh * 64 + 64],
                                in1=weff[("v", st_)][1][:, hh * 64:hh * 64 + 64])

        qkv_pass(hAG_r,
                 [weff[("q", 0)][0], weff[("q", 1)][0]],
                 [weff[("q", 0)][1], weff[("q", 1)][1]], q4w,
                 [weff[("k", 0)][0], weff[("k", 1)][0]],
                 [weff[("k", 0)][1], weff[("k", 1)][1]], k4w, make_v=True)

        perm_sb = const.tile([128, 128], BF16, tag="perm")
        nc.sync.dma_start(perm_sb[:], perm128.ap())
        cos_sb = big.tile([128, 2048], BF16, tag="cos", bufs=1)
        sin_sb = big.tile([128, 2048], BF16, tag="sin", bufs=1)
        nc.sync.dma_start(cos_sb[:], cosQ.ap())
        nc.sync.dma_start(sin_sb[:], sinQ.ap())
        q_r = big.tile([128, 2048], BF16, tag="t2048c", bufs=1, name="q_r")
        k_r = big.tile([128, 2048], BF16, tag="t2048d", bufs=1, name="k_r")
        for (src, dst) in ((q_raw, q_r), (k_raw, k_r)):
            for tg in range(4):
                sl = slice(tg * 512, (tg + 1) * 512)
                rp = psum.tile([128, 512], F32, tag="psmisc",
                               name=f"rot_ps{tg}")
                nc.tensor.matmul(rp[:], perm_sb[:], src[:, sl],
                                 start=True, stop=True)
                tmp = work.tile([128, 512], F32, tag="rope_tmp")
                nc.vector.tensor_mul(tmp[:], rp[:], sin_sb[:, sl])
                tmp2 = work.tile([128, 512], F32, tag="rope_tmp2")
                nc.vector.tensor_mul(tmp2[:], src[:, sl], cos_sb[:, sl])
                nc.vector.tensor_add(dst[:, sl], tmp[:], tmp2[:])

        # -------- Phase 3: self-attention + per-batch AllGather
        aT_b = [big.tile([128, 1024], BF16, tag=("t2048a", "t2048b")[b], bufs=1,
                         name=f"aT_b{b}") for b in range(2)]
        _attention(nc, work, psum, q_r[:], k_r[:], v_sb, aT_b,
                   n_tkblocks=8, tkb_of=lambda b, mt: 2 * mt + b)
        aAG = []
        for b in range(2):
            agi = dram.tile([128, 1024], BF16, name=f"aAG_in{b}")
            ago = dram.tile([1024, 1024], BF16, addr_space="Shared",
                            name=f"aAG{b}")
            nc.sync.dma_start(agi[:], aT_b[b][:])
            nc.gpsimd.collective_compute(
                "AllGather", ALU.bypass, replica_groups=RG,
                ins=[agi.opt()], outs=[ago.opt()])
            aAG.append(ago)

        # -------- Phase 4: out-proj + gated residual -> x1
        x1_t = _proj_residual(nc, work, psum, big, bcpool, pid, aAG, ow_sb,
                              ob_bc, x_t, (modAR, 2), "x1")

        # -------- Phase 5: cross-attention (cn_g/cn_b folded into cqw/cqb)
        hcT = work.tile([128, 8, 256], BF16, tag="hT", bufs=1, name="hcT")
        for st in range(2):
            ln = _ln_tok(nc, work, x1_t[st], eps_t)
            _transpose_to(nc, psum, ident, ln, hcT, st)
        hcAG_in = dram.tile([1024, 256], BF16)
        hcAG = dram.tile([8192, 256], BF16, addr_space="Shared")
        nc.sync.dma_start(hcAG_in[:].rearrange("(o p) t -> p o t", p=128),
                          hcT[:])
        nc.gpsimd.collective_compute(
            "AllGather", ALU.bypass, replica_groups=RG,
            ins=[hcAG_in.opt()], outs=[hcAG.opt()])

        cqw_sb = const.tile([128, 8, 128], BF16, tag="cqw")
        nc.sync.dma_start(cqw_sb[:],
                          cqwh.ap().rearrange("(o p) m -> p o m", p=128))
        cqb_sb = const.tile([128, 1], F32, tag="cqb")
        nc.sync.dma_start(cqb_sb[:], cqbh.ap())
        lq = big.tile([128, 2048], BF16, tag="t2048c", bufs=1, name="lq")
        lq4 = lq.rearrange("p (c s j) -> p c s j", c=8, s=2)
        hcAG_r = hcAG[:].rearrange("(c k p) t -> p k c t", p=128, k=8)
        qkv_pass(hcAG_r, [cqw_sb, cqw_sb], [cqb_sb, cqb_sb], lq4)

        caT_b = [big.tile([128, 1024], BF16, tag=("t2048d", "cos")[b], bufs=1,
                          name=f"caT_b{b}") for b in range(2)]
        _attention(nc, work, psum, lq[:], lk[:], lv, caT_b,
                   n_tkblocks=2, tkb_of=lambda b, mt: 2 * b + mt,
                   k_is_flat=True)
        caAG = []
        for b in range(2):
            agi = dram.tile([128, 1024], BF16, name=f"caAG_in{b}")
            ago = dram.tile([1024, 1024], BF16, addr_space="Shared",
                            name=f"caAG{b}")
            nc.sync.dma_start(agi[:], caT_b[b][:])
            nc.gpsimd.collective_compute(
                "AllGather", ALU.bypass, replica_groups=RG,
                ins=[agi.opt()], outs=[ago.opt()])
            caAG.append(ago)

        # -------- Phase 6: cow proj + residual -> x2
        x2_t = _proj_residual(nc, work, psum, big, bcpool, pid, caAG, cow_sb,
                              cob_bc, x1_t, None, "x2")
        for st in range(2):
            nc.sync.dma_start(x2_out.ap()[st * 128:(st + 1) * 128, :],
                              x2_t[st][:])

        # -------- Phase 7: h2 + logits
        h2T = work.tile([128, 8, 256], F32, tag="h2T", bufs=1)
        for st in range(2):
            ln = _ln_tok(nc, work, x2_t[st], eps_t)
            A2 = _bcast_from_mod(nc, bcpool, modAR, st, 4, True, "bcA")
            B2 = _bcast_from_mod(nc, bcpool, modAR, st, 3, False, "bcB")
            h2 = work.tile([128, 1024], F32, tag="h1")
            nc.vector.tensor_mul(h2[:], ln[:], A2[:])
            nc.vector.tensor_add(h2[:], h2[:], B2[:])
            nc.sync.dma_start(h2_out.ap()[st * 128:(st + 1) * 128, :], h2[:])
            _transpose_to(nc, psum, ident, h2, h2T, st)
        gw_sb = const.tile([128, 8, 8], F32, tag="gatew")
        nc.sync.dma_start(gw_sb[:],
                          gate_w_in.ap().rearrange("(o p) e -> p o e", p=128))
        lg_ps = psum.tile([8, 256], F32, tag="ps512", name="lg_ps")
        for k in range(8):
            nc.tensor.matmul(lg_ps[:], gw_sb[:, k, :], h2T[:, k, :],
                             start=(k == 0), stop=(k == 7))
        lg_sb = work.tile([8, 256], F32, tag="lg_sb", bufs=1)
        nc.vector.tensor_copy(lg_sb[:], lg_ps[:])
        nc.sync.dma_start(logitsT_out.ap(), lg_sb[:])

    nc.compile()
    return nc


# ---------------------------------------------------------------- launch 2
def build_launch2():
    nc = bacc.Bacc("TRN2", target_bir_lowering=False, debug=False,
                   num_devices=NC)
    xgT = nc.dram_tensor("xgT", [1024, CAP], BF16, kind="ExternalInput")
    w1 = nc.dram_tensor("w1", [1024, 2048], BF16, kind="ExternalInput")
    b1T = nc.dram_tensor("b1T", [128, 16], F32, kind="ExternalInput")
    w2 = nc.dram_tensor("w2", [2048, 1024], BF16, kind="ExternalInput")
    b2T = nc.dram_tensor("b2T", [128, 8], F32, kind="ExternalInput")
    yT_out = nc.dram_tensor("yT_out", [1024, CAP], F32, kind="ExternalOutput")

    NB = [(0, 512), (512, CAP - 512)] if CAP > 512 else [(0, CAP)]
    with tile.TileContext(nc) as tc, ExitStack() as ctx:
        big = ctx.enter_context(tc.tile_pool(name="big", bufs=1))
        work = ctx.enter_context(tc.tile_pool(name="work", bufs=3))
        psum = ctx.enter_context(tc.tile_pool(name="ps", bufs=2, space="PSUM"))

        xg_sb = big.tile([128, 8, CAP], BF16, tag="xg")
        nc.sync.dma_start(xg_sb[:], xgT.ap().rearrange("(o p) t -> p o t", p=128))
        w1_sb = big.tile([128, 8, 2048], BF16, tag="w1")
        nc.sync.dma_start(w1_sb[:], w1.ap().rearrange("(o p) m -> p o m", p=128))
        w2_sb = big.tile([128, 16, 1024], BF16, tag="w2")
        nc.sync.dma_start(w2_sb[:], w2.ap().rearrange("(o p) m -> p o m", p=128))
        b1_sb = work.tile([128, 16], F32, tag="b1", bufs=1)
        nc.sync.dma_start(b1_sb[:], b1T.ap())
        b2_sb = work.tile([128, 8], F32, tag="b2", bufs=1)
        nc.sync.dma_start(b2_sb[:], b2T.ap())
        act = big.tile([128, 16, CAP], BF16, tag="act")

        for m in range(16):
            for (n0, nn) in NB:
                pp = psum.tile([128, 512], F32, tag="ps1", name=f"ps1_{m}_{n0}")
                for k in range(8):
                    nc.tensor.matmul(
                        pp[:, :nn], w1_sb[:, k, m * 128:(m + 1) * 128],
                        xg_sb[:, k, n0:n0 + nn],
                        start=(k == 0), stop=(k == 7))
                nc.scalar.activation(
                    out=act[:, m, n0:n0 + nn], in_=pp[:, :nn], func=AF.Gelu,
                    bias=b1_sb[:, m:m + 1])
        yT_r = yT_out.ap().rearrange("(o p) t -> p o t", p=128)
        for m in range(8):
            for (n0, nn) in NB:
                pp = psum.tile([128, 512], F32, tag="ps2", name=f"ps2_{m}_{n0}")
                for k in range(16):
                    nc.tensor.matmul(
                        pp[:, :nn], w2_sb[:, k, m * 128:(m + 1) * 128],
                        act[:, k, n0:n0 + nn],
                        start=(k == 0), stop=(k == 15))
                y_sb = work.tile([128, 512], F32, tag="y_sb")
                nc.vector.tensor_scalar_add(
                    y_sb[:, :nn], pp[:, :nn], b2_sb[:, m:m + 1])
                nc.sync.dma_start(yT_r[:, m, n0:n0 + nn], y_sb[:, :nn])
    nc.compile()
    return nc


# ---------------------------------------------------------------- host glue
def _get_nc(which):
    with _lock:
        if which not in _cache:
            _cache[which] = build_launch1() if which == 1 else build_launch2()
        return _cache[which]


def _bf(a):
    import ml_dtypes
    return np.ascontiguousarray(np.asarray(a, np.float32)).astype(
        ml_dtypes.bfloat16)


def _prep_launch1_inputs(x, condition, enc, rope_cos, rope_sin, adaln_w,
                         adaln_b, qw, qb, kw, kb, vw, vb, ow, ob, cqw, cqb,
                         ckw, ckb, cvw, cvb, cow, cob, cn_g, cn_b, gate_w):
    f32 = np.float32
    x = np.asarray(x, f32)
    sc = f32(1.0 / np.sqrt(HD))
    condT = np.ascontiguousarray(np.asarray(condition, f32).T)      # [1024,2]
    encT = _bf(np.asarray(enc, f32).reshape(B * L, C).T)            # [2048,512]
    cosT = np.asarray(rope_cos, f32).T                              # [64,1024]
    sinT = np.asarray(rope_sin, f32).T
    ct2 = np.tile(cosT, (2, 1))      # [128, 1024]: rows = 2 heads x 64 dims
    st2 = np.tile(sinT, (2, 1))
    cosQ = np.empty((128, 8, 2, 128), f32)
    sinQ = np.empty((128, 8, 2, 128), f32)
    for c_ in range(8):
        for b_ in range(2):
            cosQ[:, c_, b_, :] = ct2[:, c_ * 128:(c_ + 1) * 128]
            sinQ[:, c_, b_, :] = st2[:, c_ * 128:(c_ + 1) * 128]
    cosQ = _bf(cosQ.reshape(128, 2048))
    sinQ = _bf(sinQ.reshape(128, 2048))
    # rot_half permutation: out[m] = -q[m+32] (m%64<32), +q[m-32] (else)
    perm = np.zeros((128, 128), f32)
    for h2 in range(2):
        for d in range(32):
            perm[h2 * 64 + d + 32, h2 * 64 + d] = -1.0
            perm[h2 * 64 + d, h2 * 64 + d + 32] = 1.0
    ob_bc = np.tile(np.asarray(ob, f32)[None, :], (128, 1))
    cob_bc = np.tile(np.asarray(cob, f32)[None, :], (128, 1))
    adaln_b8 = np.ascontiguousarray(
        np.tile(np.asarray(adaln_b, f32)[None, :] / NC, (2, 1)))
    qw8 = np.asarray(qw, f32) * sc
    qb8 = np.asarray(qb, f32) * sc
    # fold cross-attn LN affine (cn_g, cn_b) into cqw/cqb; also fold 1/8
    cng = np.asarray(cn_g, f32)
    cnb = np.asarray(cn_b, f32)
    cqw_f = (np.asarray(cqw, f32) * cng[:, None]) * sc
    cqb_f = (np.asarray(cqb, f32) + cnb @ np.asarray(cqw, f32)) * sc

    in_maps = []
    for c in range(NC):
        sl = slice(128 * c, 128 * (c + 1))
        xch = np.concatenate([x[0, sl], x[1, sl]], axis=0)
        in_maps.append(dict(
            xch=np.ascontiguousarray(xch),
            condTs=np.ascontiguousarray(condT[sl]),
            adaln_ws=_bf(np.asarray(adaln_w, f32)[sl]),
            adaln_b8=adaln_b8,
            qwh=_bf(qw8[:, sl]),
            qbh=np.ascontiguousarray(qb8[sl])[:, None],
            kwh=_bf(np.asarray(kw, f32)[:, sl]),
            kbh=np.ascontiguousarray(np.asarray(kb, f32)[sl])[:, None],
            vwh=_bf(np.asarray(vw, f32)[:, sl]),
            vbh_bc=np.ascontiguousarray(
                np.tile(np.asarray(vb, f32)[None, sl], (128, 1))),
            cosQ=cosQ, sinQ=sinQ, perm128=_bf(perm),
            ow_in=_bf(ow), ob_bc=ob_bc,
            encT=encT,
            cqwh=_bf(cqw_f[:, sl]),
            cqbh=np.ascontiguousarray(cqb_f[sl])[:, None],
            ckwh=_bf(np.asarray(ckw, f32)[:, sl]),
            ckbh=np.ascontiguousarray(np.asarray(ckb, f32)[sl])[:, None],
            cvwh=_bf(np.asarray(cvw, f32)[:, sl]),
            cvbh_bc=np.ascontiguousarray(
                np.tile(np.asarray(cvb, f32)[None, sl], (128, 1))),
            cow_in=_bf(cow), cob_bc=cob_bc,
            gate_w_in=np.ascontiguousarray(np.asarray(gate_w, f32)),
        ))
    return in_maps


def kernel(x, condition, enc, rope_cos, rope_sin, adaln_w, adaln_b, qw, qb,
           kw, kb, vw, vb, ow, ob, cqw, cqb, ckw, ckb, cvw, cvb, cow, cob,
           cn_g, cn_b, gate_w, ew1, eb1, ew2, eb2):
    f32 = np.float32
    trace = bool(int(os.environ.get("KERNEL_TRACE", "0")))
    if trace:
        import trace_shim
        trace_shim.install()

    in_maps = _prep_launch1_inputs(
        x, condition, enc, rope_cos, rope_sin, adaln_w, adaln_b, qw, qb, kw,
        kb, vw, vb, ow, ob, cqw, cqb, ckw, ckb, cvw, cvb, cow, cob, cn_g,
        cn_b, gate_w)
    nc1 = _get_nc(1)
    res1 = run_bass_kernel_spmd(nc1, in_maps, core_ids=list(range(NC)),
                                trace=trace)
    kernel.last_exec1 = res1.exec_time_ns

    x2 = np.empty((B, S, H), f32)
    h2 = np.empty((B, S, H), f32)
    logits = np.empty((B, S, E), f32)
    for c in range(NC):
        r = res1.results[c]
        sl = slice(128 * c, 128 * (c + 1))
        x2[0, sl] = r["x2_out"][:128]
        x2[1, sl] = r["x2_out"][128:]
        h2[0, sl] = r["h2_out"][:128]
        h2[1, sl] = r["h2_out"][128:]
        lg = r["logitsT_out"].T
        logits[0, sl] = lg[:128]
        logits[1, sl] = lg[128:]
    mod = res1.results[0]["mod_out"]
    g2 = mod[:, 5 * H:6 * H]

    # ---- routing (host, fp32)
    lg2 = logits.reshape(T, E)
    p = np.exp(lg2 - lg2.max(-1, keepdims=True))
    probs = p / p.sum(-1, keepdims=True)
    order = np.argsort(-probs, axis=-1, kind="stable")
    topi = order[:, :TOPK]
    topp = np.take_along_axis(probs, topi, axis=-1)
    topp = topp / topp.sum(-1, keepdims=True)
    cw = np.zeros((T, E), f32)
    np.put_along_axis(cw, topi, topp.astype(f32), axis=-1)

    onehot_sum = np.zeros((B, S, E), f32)
    ti = topi.reshape(B, S, TOPK)
    for kk in range(TOPK):
        for b_ in range(B):
            onehot_sum[b_, np.arange(S), ti[b_, :, kk]] += 1.0
    tokens_per_expert = onehot_sum.mean(0)
    avg_prob = probs.reshape(B, S, E).mean((0, 1))
    aux = np.float32(E * (tokens_per_expert * avg_prob).sum())

    # ---- expert-parallel FFN (launch 2)
    h2f = h2.reshape(T, H)
    tok_lists = [np.where(cw[:, e] > 0)[0] for e in range(E)]
    n_batches = max(1, max((len(t) + CAP - 1) // CAP for t in tok_lists))
    nc2 = _get_nc(2)
    ew1 = np.asarray(ew1, f32)
    ew2 = np.asarray(ew2, f32)
    eb1 = np.asarray(eb1, f32)
    eb2 = np.asarray(eb2, f32)
    w1s = [_bf(ew1[e]) for e in range(E)]
    w2s = [_bf(ew2[e]) for e in range(E)]
    b1Ts = [np.ascontiguousarray(eb1[e].reshape(16, 128).T) for e in range(E)]
    b2Ts = [np.ascontiguousarray(eb2[e].reshape(8, 128).T) for e in range(E)]
    moe = np.zeros((T, H), f32)
    kernel.last_exec2 = 0
    for bi in range(n_batches):
        maps2 = []
        batch_toks = []
        for e in range(E):
            toks = tok_lists[e][bi * CAP:(bi + 1) * CAP]
            batch_toks.append(toks)
            xg = np.zeros((CAP, H), f32)
            xg[:len(toks)] = h2f[toks]
            maps2.append(dict(xgT=_bf(xg.T), w1=w1s[e], b1T=b1Ts[e],
                              w2=w2s[e], b2T=b2Ts[e]))
        res2 = run_bass_kernel_spmd(nc2, maps2, core_ids=list(range(NC)),
                                    trace=trace)
        if trace and res2.exec_time_ns:
            kernel.last_exec2 += res2.exec_time_ns
        for e in range(E):
            toks = batch_toks[e]
            if len(toks) == 0:
                continue
            y = res2.results[e]["yT_out"].T[:len(toks)]
            moe[toks] += cw[toks, e][:, None] * y

    out = x2 + g2[:, None, :] * moe.reshape(B, S, H)
    return out.astype(f32), aux
